# revision 25
# baseline (speedup 1.0000x reference)
"""Trainium2 Bass kernel for 12-head MHA (B=2, S=4096, D=768), fp32.

Sharding: 8 cores = 2 batches x 4 head-groups (3 heads each).
Each core computes, for its (batch, 3 heads):
    Q/K/V projections, scores^T = K @ Q^T (transposed-score layout),
    exp (ScalarE, fused 1/8 scale), AV with a ones-column appended to V
    (M=65 matmul -> softmax denominator lands in PSUM row 64 for free),
    normalize, and a partial out-projection  context @ Wo_slice^T.
Host sums the 4 partial outputs per batch and adds bo.

v7 schedule (ScalarE exp is the bottleneck; everything else hides under it):
  - K(h0,h1) projections first, then attention h0 starts at ~17us; V
    chunks, remaining Q windows and all h2-only projections trickle into
    h0's exp-group stream via interleave callbacks (PE fills its slack
    between score/AV matmuls without ever stalling ACT).
  - head-major attention; out-projection emitted as fine-grained steps
    (2 matmuls or a copy+DMA per step) drained one per exp group during
    head-2's pass -- no bursty PE lumps at tile boundaries.
  - softmax denominator: DVE copy of PSUM row 64 -> SBUF->SBUF DMA to
    lane 0 -> gpsimd partition_broadcast -> reciprocal_approx_fast on
    the base-0 tile (HW rejects custom-DVE/gpsimd sources off lane 0).
  - PSUM: scores 2x[128,1024] (4 banks) + pav 2x[65,512] (2) + proj
    2x[128,512] (2, closed before h2) / po1+po2 (2, h2 pass only) = 8.
"""

import numpy as np

B, S, D = 2, 4096, 768
H, DK = 12, 64
NCORES = 8
HPC = 3                 # heads per core
DCH = D // 128          # 6 contraction chunks of 128
NT = S // 512           # 8 q-tiles / s-windows of 512
NKB = S // 128          # 32 key blocks of 128
GSZ = 2                 # k-blocks per exp group (2 PSUM banks, x2 buffers)

_CACHE = {}


def _build_bass():
    from contextlib import ExitStack

    import concourse.bass as bass  # noqa: F401
    import concourse.mybir as mybir
    import concourse.tile as tile
    from concourse import bacc

    f32 = mybir.dt.float32
    Exp = mybir.ActivationFunctionType.Exp

    nc = bacc.Bacc("TRN2", target_bir_lowering=False, debug=False)
    bf16 = mybir.dt.bfloat16

    def mm(out, lhsT, rhs, **kw):
        nc.tensor.matmul(out, lhsT=lhsT, rhs=rhs, **kw)

    xT = nc.declare_dram_parameter("xT", [D, S], bf16, isOutput=False)
    wqT = nc.declare_dram_parameter("wqT", [D, HPC * DK], bf16, isOutput=False)
    wkT = nc.declare_dram_parameter("wkT", [D, HPC * DK], bf16, isOutput=False)
    wvT = nc.declare_dram_parameter("wvT", [D, HPC * DK], bf16, isOutput=False)
    woT = nc.declare_dram_parameter("woT", [HPC * DK, D], bf16, isOutput=False)
    bq = nc.declare_dram_parameter("bq", [1, HPC * DK], bf16, isOutput=False)
    bk = nc.declare_dram_parameter("bk", [1, HPC * DK], bf16, isOutput=False)
    bv = nc.declare_dram_parameter("bv", [1, HPC * DK], bf16, isOutput=False)
    out = nc.declare_dram_parameter("out", [S, D], f32, isOutput=True)

    with tile.TileContext(nc) as tc, ExitStack() as ctx:
        const = ctx.enter_context(tc.tile_pool(name="const", bufs=1))
        pdata = ctx.enter_context(tc.tile_pool(name="pdata", bufs=1))

        ones = const.tile([1, 512], bf16, name="ones")
        nc.vector.memset(ones, 1.0)
        bq_sb = const.tile([1, HPC * DK], bf16, name="bq_sb")
        bk_sb = const.tile([1, HPC * DK], bf16, name="bk_sb")
        bv_sb = const.tile([1, HPC * DK], bf16, name="bv_sb")
        nc.sync.dma_start(out=bq_sb, in_=bq[:, :])
        nc.sync.dma_start(out=bk_sb, in_=bk[:, :])
        nc.sync.dma_start(out=bv_sb, in_=bv[:, :])

        # Persistent per-head data.
        qdup = [
            [
                pdata.tile([128, 512], bf16, name=f"qd{h}_{t}", tag=f"qd{h}_{t}")
                for t in range(NT)
            ]
            for h in range(HPC)
        ]
        kt = [
            pdata.tile([128, NKB * 64], bf16, name=f"kt{h}", tag=f"kt{h}")
            for h in range(HPC)
        ]
        vaug = [
            pdata.tile([128, NKB, 65], bf16, name=f"va{h}", tag=f"va{h}")
            for h in range(HPC)
        ]
        ctxA = [
            pdata.tile([128, 512], bf16, name=f"ctxA{t}", tag=f"ctxA{t}")
            for t in range(NT)
        ]
        ctxB = [
            pdata.tile([64, 512], bf16, name=f"ctxB{t}", tag=f"ctxB{t}")
            for t in range(NT)
        ]
        # x^T resident: one tile per 512-col window, [128, DCH, 512]
        xw = [
            pdata.tile([128, DCH, 512], bf16, name=f"xw{w}", tag=f"xw{w}")
            for w in range(NT)
        ]
        # Projection weights first (small; gate the first K matmuls), then
        # the bulk x windows in consumption order, then the out-proj weights
        # (not needed until head-2's pass).
        wq_sb = pdata.tile([128, DCH, HPC * DK], bf16, name="wq_sb")
        wk_sb = pdata.tile([128, DCH, HPC * DK], bf16, name="wk_sb")
        wv_sb = pdata.tile([128, DCH, HPC * DK], bf16, name="wv_sb")
        for wsb, wdram in ((wk_sb, wkT), (wv_sb, wvT), (wq_sb, wqT)):
            nc.sync.dma_start(
                out=wsb,
                in_=wdram.rearrange("(c p) m -> p c m", p=128),
            )

        for h in range(HPC):
            # ones column used by the AV denominator row
            nc.vector.memset(vaug[h][:, :, 64:65], 1.0)

        for w in range(NT):
            nc.sync.dma_start(
                out=xw[w],
                in_=xT[:, w * 512 : (w + 1) * 512].rearrange(
                    "(c p) n -> p c n", p=128
                ),
            )

        wo_a = pdata.tile([128, D], bf16, name="wo_a")
        wo_b = pdata.tile([64, D], bf16, name="wo_b")
        nc.sync.dma_start(out=wo_a, in_=woT[0:128, :])
        nc.sync.dma_start(out=wo_b, in_=woT[128:192, :])

        # ---------------- pools (LIFO stack discipline) -------------------
        sb2 = ctx.enter_context(tc.tile_pool(name="sb2", bufs=1))
        att_pool = tc.tile_pool(name="attp", bufs=1, space="PSUM")
        ap = att_pool.__enter__()
        proj_pool = tc.tile_pool(name="projp", bufs=1, space="PSUM")
        pp = proj_pool.__enter__()

        # ---------------- projection emitters -----------------------------
        def emit_k(w, h0, mw):
            """K^T packed into kt: even s-blocks -> partitions 0-63, odd ->
            64-127.  One [128,512] PSUM bank: cols 0:256 even, 256:512 odd.
            Only the first matmul carries start=True (bank-wide has_written
            clear); the odd half relies on per-element overwrite-where-unset
            (HW-validated)."""
            hh_list = [h0, h0 + 1] if mw == 128 else [h0]
            hsl = slice(h0 * DK, h0 * DK + mw)
            kp = pp.tile([128, 512], f32, name=f"kp{w}_{h0}", tag="proj", bufs=2)
            xw5 = xw[w].rearrange("p c (b lo n) -> p c b lo n", lo=2, n=128)
            for c in range(DCH):
                mm(kp[0:mw, 0:256], lhsT=wk_sb[:, c, hsl],
                   rhs=xw5[:, c, :, 0, :], start=(c == 0), stop=False,
                   skip_group_check=True)
                mm(kp[0:mw, 256:512], lhsT=wk_sb[:, c, hsl],
                   rhs=xw5[:, c, :, 1, :], start=False, stop=False,
                   skip_group_check=True)
            mm(kp[0:mw, :], lhsT=bk_sb[:, hsl], rhs=ones[:, :],
               start=False, stop=True, skip_group_check=True)
            wcols = slice(w * 256, (w + 1) * 256)
            for hh in hh_list:
                r0 = (hh - h0) * 64
                nc.vector.tensor_copy(kt[hh][0:64, wcols], kp[r0 : r0 + 64, 0:256])
                nc.vector.tensor_copy(
                    kt[hh][64:128, wcols], kp[r0 : r0 + 64, 256:512]
                )

        def emit_v(w, sc):
            """V natural [s-chunk, 3*64]; per-head slices copied to vaug."""
            j = w * 4 + sc
            pv = pp.tile([128, 512], f32, name=f"pv{w}_{sc}", tag="proj", bufs=2)
            for c in range(DCH):
                mm(pv[:, 0 : HPC * DK], lhsT=xw[w][:, c, sc * 128 : (sc + 1) * 128],
                   rhs=wv_sb[:, c, :], start=(c == 0), stop=False)
            mm(pv[:, 0 : HPC * DK], lhsT=ones[:, 0:128], rhs=bv_sb,
               start=False, stop=True)
            for h in range(HPC):
                nc.vector.tensor_copy(
                    vaug[h][:, j, 0:64], pv[:, h * DK : (h + 1) * DK]
                )

        def emit_q(t, h0, mw):
            hh_list = [h0, h0 + 1] if mw == 128 else [h0]
            hsl = slice(h0 * DK, h0 * DK + mw)
            pq = pp.tile([128, 512], f32, name=f"pq{t}_{h0}", tag="proj", bufs=2)
            for c in range(DCH):
                mm(pq[0:mw, :], lhsT=wq_sb[:, c, hsl], rhs=xw[t][:, c, :],
                   start=(c == 0), stop=False)
            mm(pq[0:mw, :], lhsT=bq_sb[:, hsl], rhs=ones[:, :],
               start=False, stop=True)
            for hh in hh_list:
                r0 = (hh - h0) * 64
                nc.vector.tensor_copy(qdup[hh][t][0:64, :], pq[r0 : r0 + 64, :])
                nc.vector.tensor_copy(qdup[hh][t][64:128, :], pq[r0 : r0 + 64, :])

        # ---------------- out-projection as fine-grained steps ------------
        po_pool = [None]
        op_steps = []  # deque of callables, drained one per exp group (h2)

        def push_outproj_block(scn):
            """Queue one 128-row out-proj block as 3 steps: po1 matmul pair,
            po2 matmul pair, then copies + output DMA."""
            t, sci = scn // 4, scn % 4
            ssl = slice(scn * 128, (scn + 1) * 128)
            csl = slice(sci * 128, (sci + 1) * 128)
            box = {}

            def s1():
                box["po1"] = po_pool[0].tile(
                    [128, 512], f32, name=f"po1_{scn}", tag="po1", bufs=1
                )
                mm(box["po1"], lhsT=ctxA[t][:, csl], rhs=wo_a[:, 0:512],
                   start=True, stop=False)
                mm(box["po1"], lhsT=ctxB[t][:, csl], rhs=wo_b[:, 0:512],
                   start=False, stop=True)

            def s2():
                box["po2"] = po_pool[0].tile(
                    [128, 256], f32, name=f"po2_{scn}", tag="po2", bufs=1
                )
                mm(box["po2"], lhsT=ctxA[t][:, csl], rhs=wo_a[:, 512:768],
                   start=True, stop=False)
                mm(box["po2"], lhsT=ctxB[t][:, csl], rhs=wo_b[:, 512:768],
                   start=False, stop=True)

            def s3():
                ot = sb2.tile([128, D], f32, name=f"ot{scn}", tag="ot", bufs=3)
                nc.vector.tensor_copy(ot[:, 0:512], box["po1"])
                nc.vector.tensor_copy(ot[:, 512:768], box["po2"])
                # gpsimd (SWDGE) queue: bulk output DMAs stay off the sync
                # queue that carries the latency-sensitive rc DMA
                nc.gpsimd.dma_start(out=out[ssl, :], in_=ot)

            op_steps.extend([s1, s2, s3])

        # ---------------- attention -----------------------------------------
        def attn_tile(h, t, interleave=None, drain_steps=False):
            """Scores -> exp -> AV -> normalize for one (head, q-tile).
            `interleave` maps group index -> callable (projection trickle);
            `drain_steps` pops one queued out-proj step after each group."""
            interleave = interleave or {}
            pav = ap.tile([65, 512], f32, name=f"av{t}_{h}", tag="av", bufs=2)
            for gi, g0 in enumerate(range(0, NKB, GSZ)):
                blocks = list(range(g0, g0 + GSZ))
                ps = ap.tile(
                    [128, GSZ * 512], f32,
                    name=f"sc{t}_{h}_{g0}", tag="scores", bufs=2,
                )
                for i, j in enumerate(blocks):
                    pb = (j % 2) * 64
                    col0 = (j // 4) * 256 + ((j % 4) // 2) * 128
                    mm(
                        ps[:, i * 512 : (i + 1) * 512],
                        lhsT=kt[h][pb : pb + 64, col0 : col0 + 128],
                        rhs=qdup[h][t][pb : pb + 64, :],
                        start=True, stop=True,
                    )
                et = sb2.tile(
                    [128, GSZ * 512], bf16,
                    name=f"et{t}_{h}_{g0}", tag="et", bufs=3,
                )
                nc.scalar.activation(et, ps, Exp, scale=0.125)
                for i, j in enumerate(blocks):
                    mm(
                        pav,
                        lhsT=vaug[h][:, j, :],
                        rhs=et[:, i * 512 : (i + 1) * 512],
                        start=(j == 0), stop=(j == NKB - 1),
                    )
                if gi in interleave:
                    interleave[gi]()
                if drain_steps and op_steps:
                    op_steps.pop(0)()
            # normalize (see docstring for the lane/base-partition rules)
            den = sb2.tile([65, 512], f32, name=f"den{t}_{h}", tag="den", bufs=2)
            nc.vector.tensor_copy(den[64:65, :], pav[64:65, :])
            rc = sb2.tile([1, 512], f32, name=f"rc{t}_{h}", tag="rc", bufs=2)
            nc.sync.dma_start(out=rc, in_=den[64:65, :])
            dbc = sb2.tile([64, 512], f32, name=f"dbc{t}_{h}", tag="dbc", bufs=2)
            nc.gpsimd.partition_broadcast(dbc, rc, channels=64)
            bc = sb2.tile([64, 512], f32, name=f"bc{t}_{h}", tag="bc", bufs=2)
            nc.vector.reciprocal_approx_fast(out=bc, in_=dbc)
            if h == 0:
                dst = ctxA[t][0:64, :]
            elif h == 1:
                dst = ctxA[t][64:128, :]
            else:
                dst = ctxB[t][0:64, :]
            nc.vector.tensor_mul(dst, pav[0:64, :], bc)

        # ---------------- emission schedule -------------------------------
        # Startup: K for the head pair (gates all h0 scores), Q(t0), V(w0).
        with nc.named_scope("proj_k01"):
            for w in range(NT):
                emit_k(w, 0, 128)
        with nc.named_scope("proj_start"):
            emit_q(0, 0, 128)
            for sc in range(4):
                emit_v(0, sc)

        # h0's pass carries the V trickle (2 chunks/group, 2 groups ahead),
        # the remaining Q(h0,h1) windows, and all h2-only projections.
        def h0_interleave(t):
            il = {}
            if t == 0:
                for g in range(14):
                    j0, j1 = 2 * g + 4, 2 * g + 5
                    il[g] = (lambda a=j0, b=j1:
                             (emit_v(a // 4, a % 4), emit_v(b // 4, b % 4)))
                il[14] = lambda: emit_q(1, 0, 128)
                il[15] = lambda: emit_k(0, 2, 64)
            elif t < 7:
                nxt = {1: (1, 2), 2: (3, 4), 3: (5, 6), 4: (7, None)}.get(t)
                il[2] = lambda tq=t + 1: emit_q(tq, 0, 128)
                if nxt is not None:
                    a, b = nxt
                    il[5] = lambda w=a: emit_k(w, 2, 64)
                    if b is not None:
                        il[11] = lambda w=b: emit_k(w, 2, 64)
                else:
                    il[5] = lambda tq=2 * t - 9: emit_q(tq, 2, 64)
                    il[11] = lambda tq=2 * t - 8: emit_q(tq, 2, 64)
            else:
                il[2] = lambda: emit_q(5, 2, 64)
                il[5] = lambda: emit_q(6, 2, 64)
                il[11] = lambda: emit_q(7, 2, 64)
            return il

        with nc.named_scope("attn_h0"):
            for t in range(NT):
                attn_tile(0, t, interleave=h0_interleave(t))
        with nc.named_scope("attn_h1"):
            for t in range(NT):
                il = {}
                if t == 0:
                    # the one h2 Q window not covered during h0's pass
                    il = {2: lambda: emit_q(0, 2, 64)}
                attn_tile(1, t, interleave=il)

        # projections all consumed; swap proj banks for the out-proj banks
        proj_pool.__exit__(None, None, None)
        po_ctx = tc.tile_pool(name="pop", bufs=1, space="PSUM")
        po_pool[0] = po_ctx.__enter__()

        with nc.named_scope("attn_h2"):
            for t in range(NT):
                if t > 0:
                    for k in range(4):
                        push_outproj_block((t - 1) * 4 + k)
                attn_tile(2, t, drain_steps=True)
        with nc.named_scope("outproj_tail"):
            for k in range(4):
                push_outproj_block(7 * 4 + k)
            while op_steps:
                op_steps.pop(0)()
        po_ctx.__exit__(None, None, None)
        att_pool.__exit__(None, None, None)

    nc.compile()
    return nc


def _get_nc():
    if "nc" not in _CACHE:
        _CACHE["nc"] = _build_bass()
    return _CACHE["nc"]


def make_in_maps(x, Wq, bq, Wk, bk, Wv, bv, Wo, bo):
    """Per-core input dicts (host-side sharding + layout prep, bf16 cast)."""
    import ml_dtypes

    bf = ml_dtypes.bfloat16
    x = np.asarray(x, dtype=np.float32)
    in_maps = []
    for c in range(NCORES):
        b = c // 4
        h0 = (c % 4) * HPC
        rows = slice(h0 * DK, (h0 + HPC) * DK)
        in_maps.append(
            {
                "xT": np.ascontiguousarray(x[b].T).astype(bf),
                "wqT": np.ascontiguousarray(np.asarray(Wq)[rows, :].T).astype(bf),
                "wkT": np.ascontiguousarray(np.asarray(Wk)[rows, :].T).astype(bf),
                "wvT": np.ascontiguousarray(np.asarray(Wv)[rows, :].T).astype(bf),
                "woT": np.ascontiguousarray(np.asarray(Wo)[:, rows].T).astype(bf),
                "bq": np.asarray(bq, dtype=np.float32)[rows][None, :].astype(bf),
                "bk": np.asarray(bk, dtype=np.float32)[rows][None, :].astype(bf),
                "bv": np.asarray(bv, dtype=np.float32)[rows][None, :].astype(bf),
            }
        )
    return in_maps


def kernel(x, Wq, bq, Wk, bk, Wv, bv, Wo, bo, _trace=False):
    from concourse.bass_utils import run_bass_kernel_spmd

    nc = _get_nc()
    in_maps = make_in_maps(x, Wq, bq, Wk, bk, Wv, bv, Wo, bo)
    res = run_bass_kernel_spmd(
        nc, in_maps, core_ids=list(range(NCORES)), trace=_trace
    )
    _CACHE["last_results"] = res
    out = np.zeros((B, S, D), dtype=np.float32)
    for c in range(NCORES):
        out[c // 4] += res.results[c]["out"]
    out += np.asarray(bo, dtype=np.float32)[None, None, :]
    return out
